# revision 13
# baseline (speedup 1.0000x reference)
"""Trainium2 Bass kernel for CapsuleLayer (nn_CapsuleLayer_45552423142009).

Computes, for x[B,768]:
  u = squash(x @ Wp + bp)            # [B, 8, 16]  (squash over last dim)
  u_hat[b,p,c,:] = u[b,p,:] @ W[p,c] # [B, 8, 5, 16]
  3 iterations of dynamic routing -> v [B, 5, 16]

Strategy: pure data-parallel over 8 NeuronCores (batch sharded 16384/core).
On-chip layout is "transposed": features on partitions, batch on the free
dim (512-wide tiles).  PE does transposes of x, the two big matmuls and all
broadcast / segment-sum reductions (via 0/1 selector matrices, fp32r at
1 cycle/row).  DVE/ACT/GPSIMD do the elementwise work.
"""

import sys
import numpy as np

sys.path.insert(0, "/opt/trn_rl_repo")

from concourse import bass, bacc, mybir  # noqa: E402
from concourse import tile  # noqa: E402
from concourse.bass_utils import run_bass_kernel_spmd  # noqa: E402
from concourse.alu_op_type import AluOpType  # noqa: E402

F32 = mybir.dt.float32
F32R = mybir.dt.float32r
AF = mybir.ActivationFunctionType

B = 131072
D = 768
P = 8
PD = 16
C = 5
CD = 16
NCORES = 8
BC = B // NCORES          # 16384 batch rows per core
NB = 512                  # batch columns per tile
NT = BC // NB             # 32 tiles

# selector blob column offsets
SEL_SSEL8 = 0      # [128, 8]   sum o-groups of 16 -> p
SEL_PSEL16 = 8     # [128, 16]  0.2 * (sum over p at fixed j)
SEL_PSEL8 = 24     # [128, 8]   sum over j at fixed p
SEL_IDENT = 32     # [128, 128] identity
SEL_TILE8 = 160    # [16, 128]  broadcast j -> (p, j)
SEL_SBC = 288      # [8, 128]   broadcast p -> (p, o)
SEL_JSEL = 416     # [80, 5]    sum over j at fixed c
SEL_JBC = 421      # [5, 80]    broadcast c -> (c, j)
SEL_CSEL = 501     # [40, 8]    sum over c at fixed p   (logits layout (c,p))
SEL_CBC = 509      # [8, 40]    broadcast p -> (c, p)
SEL_BSEL = 549     # [40, 640]  5 x [40,128]: broadcast (c,p) -> (p,i) for class c
SEL_ESEL = 1190    # [16, 40]   5 x [16,8]: col c ones (vsq accumulate)
SEL_GBC = 1230     # [8, 80]    5 x [8,16]: row c ones (g -> j-bcast, class c)
SEL_GBC40 = 1310   # [8, 40]    [c', (c,p)] = d_c'c (g -> (c,p) bcast)
SEL_ASEL = 1350    # [128, 200] 5 x [128,40]: [(p,j),(c',p')] = d_pp' d_c'c
SEL_W = 1550
CST_W = SEL_W + 768 + 640 + 80 + 1


def _r(ap):
    return ap.bitcast(F32R)


def build_selectors() -> np.ndarray:
    sel = np.zeros((128, SEL_W), dtype=np.float32)
    for m in range(128):
        sel[m, SEL_SSEL8 + m // 16] = 1.0                      # Ssel8
    for p in range(P):
        for j in range(PD):
            sel[p * 16 + j, SEL_PSEL16 + j] = 0.2              # Psel16 (x0.2)
            sel[p * 16 + j, SEL_PSEL8 + p] = 1.0               # Psel8
    sel[:, SEL_IDENT:SEL_IDENT + 128] = np.eye(128, dtype=np.float32)
    for j in range(16):
        for p in range(P):
            sel[j, SEL_TILE8 + p * 16 + j] = 1.0               # Tile8
    for p in range(P):
        sel[p, SEL_SBC + p * 16:SEL_SBC + (p + 1) * 16] = 1.0  # Sbc
    for c in range(C):
        for j in range(CD):
            sel[c * 16 + j, SEL_JSEL + c] = 1.0                # Jsel
            sel[c, SEL_JBC + c * 16 + j] = 1.0                 # Jbc
    for c in range(C):
        for p in range(P):
            sel[c * 8 + p, SEL_CSEL + p] = 1.0                 # Csel
            sel[p, SEL_CBC + c * 8 + p] = 1.0                  # Cbc
            # Bsel_c: [(c',p), (p',i)] = d_cc' d_pp'
            sel[c * 8 + p, SEL_BSEL + c * 128 + p * 16:
                SEL_BSEL + c * 128 + (p + 1) * 16] = 1.0
    for c in range(C):
        for j in range(CD):
            sel[j, SEL_ESEL + c * 8 + c] = 1.0                 # Esel_c col c
            sel[c, SEL_GBC + c * 16 + j] = 1.0                 # Gbc_c row c
    for c in range(C):
        for p in range(P):
            sel[c, SEL_GBC40 + c * 8 + p] = 1.0                # Gbc40
    for c in range(C):
        for p in range(P):
            for j in range(CD):
                # Asel_c: [(p,j), (c',p')] = d_pp' d_c'c
                sel[p * 16 + j, SEL_ASEL + c * 40 + c * 8 + p] = 1.0
    return sel


def build_nc(nt: int = NT) -> bass.Bass:
    bc = nt * NB
    nc = bacc.Bacc(None)

    x_d = nc.declare_dram_parameter("xc", [bc, D], F32R, isOutput=False)
    cst_d = nc.declare_dram_parameter("cst", [128, CST_W], F32R, isOutput=False)
    v_d = nc.declare_dram_parameter("vout", [bc, C * CD], F32, isOutput=True)

    with tile.TileContext(nc) as tc, nc.allow_low_precision(reason="float32r matmul inputs"):
        with (
            tc.sbuf_pool(name="const", bufs=1) as cpool,
            tc.sbuf_pool(name="xin", bufs=2) as xpool,
            tc.sbuf_pool(name="xt", bufs=2) as xtpool,
            tc.sbuf_pool(name="mid", bufs=2) as mpool,
            tc.sbuf_pool(name="uh", bufs=2) as uhpool,
            tc.sbuf_pool(name="rt", bufs=2) as rtpool,
            tc.sbuf_pool(name="sm", bufs=3) as smpool,
            tc.psum_pool(name="pxt", bufs=1) as pxt,
            tc.psum_pool(name="puh", bufs=2) as puhp,
            tc.psum_pool(name="pbc", bufs=2) as pbcp,
            tc.psum_pool(name="pmid", bufs=1) as pmidp,
            tc.psum_pool(name="psm", bufs=2) as psmp,
        ):
            # ---- load constants (one DMA), then stage through DVE so every
            # consumer depends on the DVE semaphore (merges with data deps;
            # walrus allows only ~2 distinct sync waits per instruction) ----
            cst0 = cpool.tile([128, CST_W], F32R)
            nc.sync.dma_start(out=cst0[:], in_=cst_d[:])
            cst = cpool.tile([128, CST_W], F32R)
            nc.vector.tensor_copy(cst[:], cst0[:])
            sel_sb = cst[:, 0:SEL_W]
            wp_sb = cst[:, SEL_W:SEL_W + 768]
            wbd_sb = cst[:, SEL_W + 768:SEL_W + 1408]
            wflat_sb = cst[:, SEL_W + 1408:SEL_W + 1488]
            bp_sb = cst[:, SEL_W + 1488:SEL_W + 1489].bitcast(F32)

            ident = sel_sb[:, SEL_IDENT:SEL_IDENT + 128]

            for it in range(nt):
                # ---- load x tile [512, 768] as 4 x [128, 768] ----
                x_sb = xpool.tile([128, 4, 768], F32R, tag="xin")
                src = x_d[it * NB:(it + 1) * NB, :].rearrange(
                    "(q p) d -> p q d", p=128)
                nc.sync.dma_start(out=x_sb[:], in_=src)

                # ---- transpose x -> xT chunks [128(d), 512(b)] x 6 ----
                xT = xtpool.tile([128, 6, NB], F32R, tag="xt")
                for k in range(6):
                    pt = pxt.tile([128, NB], F32R, tag="pxt")
                    for q in range(4):
                        nc.tensor.transpose(
                            _r(pt[:, q * 128:(q + 1) * 128]),
                            _r(x_sb[:, q, k * 128:(k + 1) * 128]),
                            _r(ident),
                        )
                    nc.vector.tensor_copy(xT[:, k, :], pt[:])

                # ---- mm1: u_pre[(p,o), b] = Wp^T x^T  (+bias via ACT) ----
                pu = pmidp.tile([128, NB], F32, tag="pmid")
                for k in range(6):
                    nc.tensor.matmul(
                        pu[:], _r(wp_sb[:, k * 128:(k + 1) * 128]),
                        _r(xT[:, k, :]), start=(k == 0), stop=(k == 5))
                u_pre = mpool.tile([128, NB], F32, tag="mid")
                nc.scalar.activation(u_pre[:], pu[:], AF.Identity,
                                     bias=bp_sb[:], scale=1.0)

                # ---- squash factor f[p, b] ----
                usq = mpool.tile([128, NB], F32R, tag="mid2")
                nc.vector.tensor_mul(usq[:], u_pre[:], u_pre[:])
                psq = psmp.tile([8, NB], F32, tag="psm")
                nc.tensor.matmul(psq[:], _r(sel_sb[:, SEL_SSEL8:SEL_SSEL8 + 8]),
                                 _r(usq[:]), start=True, stop=True)
                srt = smpool.tile([8, NB], F32, tag="sm")
                nc.scalar.sqrt(srt[:], psq[:])
                sq1 = smpool.tile([8, NB], F32, tag="sm")
                nc.scalar.add(sq1[:], psq[:], 1.0)
                den = smpool.tile([8, NB], F32, tag="sm")
                # den = (srt + 1e-8) * sq1
                nc.vector.scalar_tensor_tensor(
                    den[:], srt[:], 1e-8, sq1[:],
                    op0=AluOpType.add, op1=AluOpType.mult)
                rden = smpool.tile([8, NB], F32, tag="sm")
                nc.vector.reciprocal(rden[:], den[:])
                fz = smpool.tile([8, NB], F32R, tag="sm")
                nc.vector.tensor_mul(fz[:], psq[:], rden[:])
                pfb = pbcp.tile([128, NB], F32, tag="pbc")
                nc.tensor.matmul(pfb[:], _r(sel_sb[:8, SEL_SBC:SEL_SBC + 128]),
                                 _r(fz[:]), start=True, stop=True)
                u = mpool.tile([128, NB], F32R, tag="mid3")
                nc.vector.tensor_mul(u[:], u_pre[:], pfb[:])

                # ---- u_hat_c = Wbd_c^T u   (5 psum banks -> sbuf) ----
                uh = []
                for c in range(C):
                    puh = puhp.tile([128, NB], F32, tag="puh")
                    nc.tensor.matmul(
                        puh[:], _r(wbd_sb[:, c * 128:(c + 1) * 128]),
                        _r(u[:]), start=True, stop=True)
                    uhc = uhpool.tile([128, NB], F32R, tag=f"uh{c}")
                    if c % 2 == 0:
                        nc.scalar.copy(uhc[:], puh[:])
                    else:
                        nc.vector.tensor_copy(uhc[:], puh[:])
                    uh.append(uhc)

                # ---- routing ----
                logit = None
                v5 = None
                for itr in range(3):
                    if itr > 0:
                        e = rtpool.tile([40, NB], F32R, tag="rt_e")
                        nc.scalar.activation(e[:], logit[:], AF.Exp)
                        pden = psmp.tile([8, NB], F32, tag="psm")
                        nc.tensor.matmul(
                            pden[:], _r(sel_sb[:40, SEL_CSEL:SEL_CSEL + 8]),
                            _r(e[:]), start=True, stop=True)
                        rd = smpool.tile([8, NB], F32R, tag="sm")
                        nc.vector.reciprocal(rd[:], pden[:])
                        pdb = pbcp.tile([40, NB], F32, tag="pbc")
                        nc.tensor.matmul(
                            pdb[:], _r(sel_sb[:8, SEL_CBC:SEL_CBC + 40]),
                            _r(rd[:]), start=True, stop=True)
                        cn = rtpool.tile([40, NB], F32R, tag="rt_cn")
                        nc.vector.tensor_mul(cn[:], e[:], pdb[:])

                    # s[j, c, b] per class via matmul; copy to sbuf (rounded)
                    s_sb = rtpool.tile([16, 5, NB], F32R, tag="rt_s")
                    for c in range(C):
                        psc = psmp.tile([16, NB], F32, tag="psm")
                        if itr == 0:
                            nc.tensor.matmul(
                                psc[:],
                                _r(sel_sb[:, SEL_PSEL16:SEL_PSEL16 + 16]),
                                _r(uh[c][:]), start=True, stop=True)
                        else:
                            pcb = pbcp.tile([128, NB], F32, tag="pbc")
                            nc.tensor.matmul(
                                pcb[:],
                                _r(sel_sb[:40, SEL_BSEL + c * 128:
                                          SEL_BSEL + (c + 1) * 128]),
                                _r(cn[:]), start=True, stop=True)
                            t = rtpool.tile([128, NB], F32R, tag="rt_t")
                            nc.vector.tensor_mul(t[:], u[:], pcb[:])
                            nc.tensor.matmul(
                                psc[:],
                                _r(wflat_sb[:, c * 16:(c + 1) * 16]),
                                _r(t[:]), start=True, stop=True)
                        nc.scalar.copy(s_sb[:, c, :], psc[:])

                    # vsq[c, b] = sum_j s^2 via accumulating one-hot matmuls
                    ssq = rtpool.tile([16, 5, NB], F32R, tag="rt_ssq")
                    nc.vector.tensor_mul(ssq[:], s_sb[:], s_sb[:])
                    pvq = psmp.tile([8, NB], F32, tag="psm")
                    for c in range(C):
                        nc.tensor.matmul(
                            pvq[:], _r(sel_sb[:16, SEL_ESEL + c * 8:
                                              SEL_ESEL + (c + 1) * 8]),
                            _r(ssq[:, c, :]), start=(c == 0), stop=(c == 4))
                    # g = vsq / ((1+vsq) (sqrt(vsq)+1e-8))
                    vsrt = smpool.tile([8, NB], F32, tag="sm")
                    nc.scalar.sqrt(vsrt[:], pvq[:])
                    vsq1 = smpool.tile([8, NB], F32, tag="sm")
                    nc.scalar.add(vsq1[:], pvq[:], 1.0)
                    vden = smpool.tile([8, NB], F32, tag="sm")
                    nc.vector.scalar_tensor_tensor(
                        vden[:], vsrt[:], 1e-8, vsq1[:],
                        op0=AluOpType.add, op1=AluOpType.mult)
                    rvd = smpool.tile([8, NB], F32, tag="sm")
                    nc.vector.reciprocal(rvd[:], vden[:])
                    g = smpool.tile([8, NB], F32R, tag="sm")
                    nc.vector.tensor_mul(g[:], pvq[:], rvd[:])

                    if itr < 2:
                        # agreement with v = g*s folded after the j-sum:
                        # atil[(c,p), b] = sum_j uh_c[(p,j),b] * s[j,c,b]
                        pat = pmidp.tile([40, NB], F32, tag="pmid")
                        for c in range(C):
                            pvb = pbcp.tile([128, NB], F32, tag="pbc")
                            nc.tensor.matmul(
                                pvb[:],
                                _r(sel_sb[:16, SEL_TILE8:SEL_TILE8 + 128]),
                                _r(s_sb[:, c, :]), start=True, stop=True)
                            pr = rtpool.tile([128, NB], F32R, tag="rt_pr")
                            nc.vector.tensor_mul(pr[:], uh[c][:], pvb[:])
                            nc.tensor.matmul(
                                pat[:],
                                _r(sel_sb[:, SEL_ASEL + c * 40:
                                          SEL_ASEL + (c + 1) * 40]),
                                _r(pr[:]), start=(c == 0), stop=(c == 4))
                        ats = rtpool.tile([40, NB], F32, tag="rt_ats")
                        nc.scalar.copy(ats[:], pat[:])
                        pg40 = psmp.tile([40, NB], F32, tag="psm")
                        nc.tensor.matmul(
                            pg40[:], _r(sel_sb[:8, SEL_GBC40:SEL_GBC40 + 40]),
                            _r(g[:]), start=True, stop=True)
                        if itr == 0:
                            logit = rtpool.tile([40, NB], F32, tag="rt_lg")
                            nc.vector.tensor_mul(logit[:], ats[:], pg40[:])
                        else:
                            a40 = rtpool.tile([40, NB], F32, tag="rt_a40")
                            nc.vector.tensor_mul(a40[:], ats[:], pg40[:])
                            lg2 = rtpool.tile([40, NB], F32, tag="rt_lg2")
                            nc.vector.tensor_add(lg2[:], logit[:], a40[:])
                            logit = lg2
                    else:
                        # final v[j, c, b] = s * g_bcast
                        v5 = rtpool.tile([16, 5, NB], F32R, tag="rt_v")
                        for c in range(C):
                            pgb = psmp.tile([16, NB], F32, tag="psm")
                            nc.tensor.matmul(
                                pgb[:], _r(sel_sb[:8, SEL_GBC + c * 16:
                                                  SEL_GBC + (c + 1) * 16]),
                                _r(g[:]), start=True, stop=True)
                            nc.vector.tensor_mul(
                                v5[:, c, :], s_sb[:, c, :], pgb[:])

                # ---- transpose v back to [b, (c,j)] and store ----
                vo = rtpool.tile([128, 4, 80], F32, tag="rt_vo")
                for q in range(4):
                    pvt = pbcp.tile([128, 80], F32R, tag="pbc")
                    for c in range(C):
                        nc.tensor.transpose(
                            _r(pvt[:, c * 16:(c + 1) * 16]),
                            _r(v5[:, c, q * 128:(q + 1) * 128]),
                            _r(sel_sb[:16, SEL_IDENT:SEL_IDENT + 16]))
                    if q % 2 == 0:
                        nc.scalar.copy(vo[:, q, :], pvt[:])
                    else:
                        nc.vector.tensor_copy(vo[:, q, :], pvt[:])
                dst = v_d[it * NB:(it + 1) * NB, :].rearrange(
                    "(q p) j -> p q j", p=128)
                nc.sync.dma_start(out=dst, in_=vo[:])

    nc.compile()
    return nc


_NC_CACHE: dict = {}


def _get_nc(nt: int) -> bass.Bass:
    if nt not in _NC_CACHE:
        _NC_CACHE[nt] = build_nc(nt)
    return _NC_CACHE[nt]


def _prep_weights(Wp, bp, W):
    Wp = np.asarray(Wp, np.float32)
    bp = np.asarray(bp, np.float32)
    W = np.asarray(W, np.float32)
    wp_flat = Wp.transpose(1, 0, 2).reshape(768, 128)          # [d, (p,o)]
    wp_h = np.ascontiguousarray(
        wp_flat.reshape(6, 128, 128).transpose(1, 0, 2).reshape(128, 768))
    wbd_h = np.zeros((128, 5, 128), np.float32)
    for p in range(P):
        wbd_h[p * 16:(p + 1) * 16, :, p * 16:(p + 1) * 16] = \
            W[p].transpose(1, 0, 2)                            # [i, c, j]
    wbd_h = np.ascontiguousarray(wbd_h.reshape(128, 640))
    wflat_h = np.ascontiguousarray(
        W.transpose(0, 2, 1, 3).reshape(128, 5 * 16))          # [(p,i), (c,j)]
    bp_h = np.ascontiguousarray(bp.reshape(128, 1))
    sel_h = build_selectors()
    return wp_h, wbd_h, wflat_h, bp_h, sel_h


def pack_consts(Wp, bp, W):
    wp_h, wbd_h, wflat_h, bp_h, sel_h = _prep_weights(Wp, bp, W)
    cst = np.concatenate([sel_h, wp_h, wbd_h, wflat_h, bp_h], axis=1)
    assert cst.shape == (128, CST_W), cst.shape
    return np.ascontiguousarray(cst)


def kernel(x, Wp, bp, W):
    x = np.asarray(x, np.float32)
    cst = pack_consts(Wp, bp, W)
    nc = _get_nc(NT)
    in_maps = [{"xc": np.ascontiguousarray(x[i * BC:(i + 1) * BC]), "cst": cst}
               for i in range(NCORES)]
    res = run_bass_kernel_spmd(nc, in_maps, list(range(NCORES)))
    out = np.concatenate([res.results[i]["vout"] for i in range(NCORES)], axis=0)
    return out.reshape(B, C, CD)
